# revision 3
# baseline (speedup 1.0000x reference)
"""Multi-head self-attention (B=2, T=2048, D=1024, H=16) on 8 TRN2 NeuronCores.

Sharding: heads {2c, 2c+1} (both batches) on core c -> attention head-parallel;
two 8-core AllToAlls (one per local head) redistribute attention output from
head-slices to row-blocks; out_proj row-sharded (core c computes flattened rows
[512c, 512c+512) of the [4096, 1024] output). w_o rows are host-permuted to
match the AllToAll output dim order.

All matmuls run in float32r (TF32-like: ~1.3e-4 rel err, 4x faster than fp32).
Softmax uses the transposed-scores layout [s, q]: exp on ACT (no max
subtraction needed in fp32 range), row-sums via a ones-column appended to V in
the PV matmul, reciprocal via exp(-ln r) on ACT, normalizer applied through a
PE outer-product broadcast + DVE multiply.

Attention interleaves the two batches' instances of the same local head
(independent chains) to keep TensorE dense (avoids HAM re-throttle), with PV
lagged one s-chunk behind exp. The first AllToAll overlaps the second head's
attention.
"""
import os
import numpy as np

B, T, D, H = 2, 2048, 1024, 16
HD = D // H
SCALE = HD ** -0.5
NC = 8
HPC = H // NC          # heads per core = 2
ROWS = B * T // NC     # output rows per core = 512

LAST_EXEC_TIME_NS = None
_CACHE = {}


def _install_ntff_hook():
    """Register the axon NTFF profile hook (missing antenv.axon_hooks shim)
    so run_bass_kernel_spmd(trace=True) can return exec_time_ns."""
    import sys
    import types
    try:
        import antenv
        if "antenv.axon_hooks" in sys.modules:
            return
        mod = types.ModuleType("antenv.axon_hooks")
        state = {"hook": None}
        mod.set_axon_ntff_profile_hook = lambda h: state.__setitem__("hook", h)
        mod.get_axon_ntff_profile_hook = lambda: state["hook"]
        sys.modules["antenv.axon_hooks"] = mod
        antenv.axon_hooks = mod
        from trn_agent_boot.trn_boot import _ntff_profile_via_ctypes
        mod.set_axon_ntff_profile_hook(
            _ntff_profile_via_ctypes("/opt/axon/libaxon_pjrt.so")
        )
    except Exception:
        pass


def _install_ntff_hook():
    """Register the axon NTFF profile hook (missing antenv.axon_hooks shim)
    so run_bass_kernel_spmd(trace=True) can return exec_time_ns."""
    import sys
    import types
    try:
        import antenv
        if "antenv.axon_hooks" in sys.modules:
            return
        mod = types.ModuleType("antenv.axon_hooks")
        state = {"hook": None}
        mod.set_axon_ntff_profile_hook = lambda h: state.__setitem__("hook", h)
        mod.get_axon_ntff_profile_hook = lambda: state["hook"]
        sys.modules["antenv.axon_hooks"] = mod
        antenv.axon_hooks = mod
        from trn_agent_boot.trn_boot import _ntff_profile_via_ctypes
        mod.set_axon_ntff_profile_hook(
            _ntff_profile_via_ctypes("/opt/axon/libaxon_pjrt.so")
        )
    except Exception:
        pass


def _build():
    import concourse.bass as bass
    import concourse.tile as tile
    from concourse import bacc, mybir

    F32 = mybir.dt.float32
    F32R = mybir.dt.float32r
    AFT = mybir.ActivationFunctionType

    nc = bacc.Bacc(
        "TRN2", target_bir_lowering=False, debug=False,
        enable_asserts=True, num_devices=NC,
    )

    # ---- I/O ----
    qT_in = nc.dram_tensor("qT", [B, D, T], F32R, kind="ExternalInput")
    wqkv_in = nc.dram_tensor("wqkv", [D, 3 * 128], F32R, kind="ExternalInput")
    bqkv_in = nc.dram_tensor("bqkv", [128, 3], F32, kind="ExternalInput")
    # wo rows permuted: [even-head dims (512), odd-head dims (512)]
    wo_in = nc.dram_tensor("wo", [D, D], F32R, kind="ExternalInput")
    wob_in = nc.dram_tensor("wob", [1, D], F32R, kind="ExternalInput")
    iden_in = nc.dram_tensor("iden", [128, 128], F32R, kind="ExternalInput")
    out = nc.dram_tensor("out", [ROWS, D], F32, kind="ExternalOutput")

    # A2A bounce buffers, one pair per local head
    a2a_in = [nc.dram_tensor(f"a2a_in{h}", [NC, HD, ROWS], F32R)
              for h in range(HPC)]
    a2a_out = [nc.dram_tensor(f"a2a_out{h}", [NC, HD, ROWS], F32R)
               for h in range(HPC)]

    with tile.TileContext(nc) as tc:
        with (
            tc.tile_pool(name="persist", bufs=1) as persist,
            tc.tile_pool(name="qrhs", bufs=3) as qrhs_pool,
            tc.tile_pool(name="vt", bufs=1) as vt_pool,
            tc.tile_pool(name="exp", bufs=5) as exp_pool,
            tc.tile_pool(name="unnorm", bufs=5) as unnorm_pool,
            tc.tile_pool(name="scaled", bufs=2) as scaled_pool,
            tc.tile_pool(name="rstage", bufs=2) as rstage_pool,
            tc.tile_pool(name="rinv", bufs=2) as rinv_pool,
            tc.tile_pool(name="oplhs", bufs=4) as oplhs_pool,
            tc.tile_pool(name="woc", bufs=2) as woc_pool,
            tc.tile_pool(name="fin", bufs=2) as fin_pool,
        ):
            # ---- persistent tiles ----
            iden = persist.tile([128, 128], F32R, tag="iden")
            nc.sync.dma_start(iden[:], iden_in[:, :])
            ones = persist.tile([1, 128], F32R, tag="ones")
            nc.any.memset(ones[:].bitcast(F32), 1.0)
            bias_sb = persist.tile([128, 3], F32, tag="bias")
            nc.sync.dma_start(bias_sb[:], bqkv_in[:, :])
            wqkv_sb = persist.tile([128, 8, 3 * 128], F32R, tag="wqkv")
            nc.sync.dma_start(
                wqkv_sb[:], wqkv_in.ap().rearrange("(kc p) m -> p kc m", p=128)
            )
            wob_sb = persist.tile([1, D], F32R, tag="wob")
            nc.sync.dma_start(wob_sb[:], wob_in[:, :])

            q_sb = [persist.tile([128, T], F32R, tag=f"q{b}", name=f"q_sb{b}")
                    for b in range(B)]
            k_sb = [persist.tile([128, T], F32R, tag=f"k{b}", name=f"k_sb{b}")
                    for b in range(B)]
            v_aug = [
                persist.tile([128, 16, HD + 1], F32R, tag=f"va{i}",
                             name=f"v_aug{i}")
                for i in range(B * HPC)
            ]

            # ================= Phase A: projections =================
            with (
                tc.tile_pool(name="px", bufs=4, space="PSUM") as px_pool,
                tc.tile_pool(name="pt", bufs=2, space="PSUM") as pt_pool,
            ):
                for b in range(B):
                    vT = vt_pool.tile([128, T], F32R, tag="vt")
                    for nb in range(4):
                        ps = [
                            px_pool.tile([128, 512], F32, tag="px",
                                         name=f"ps{b}_{nb}_{mb}")
                            for mb in range(3)
                        ]
                        for kc in range(8):
                            qr = qrhs_pool.tile([128, 512], F32R, tag="qr")
                            nc.sync.dma_start(
                                qr[:],
                                qT_in[b, kc * 128:(kc + 1) * 128,
                                      nb * 512:(nb + 1) * 512],
                            )
                            for mb in range(3):
                                nc.tensor.matmul(
                                    ps[mb][:],
                                    wqkv_sb[:, kc, mb * 128:(mb + 1) * 128],
                                    qr[:],
                                    start=(kc == 0), stop=(kc == 7),
                                )
                        dests = [q_sb[b], k_sb[b], vT]
                        for mb in range(3):
                            nc.vector.tensor_scalar_add(
                                dests[mb][:, nb * 512:(nb + 1) * 512],
                                ps[mb][:],
                                bias_sb[:, mb:mb + 1],
                            )
                        # bf16 residual of q: q_lo = (psum + b_q) - bf16(q)
                        nc.vector.scalar_tensor_tensor(
                            q_lo[b][:, nb * 512:(nb + 1) * 512],
                            ps[0][:],
                            bias_sb[:, 0:1],
                            q_sb[b][:, nb * 512:(nb + 1) * 512],
                            mybir.AluOpType.add,
                            mybir.AluOpType.subtract,
                        )
                    # transpose vT -> v_aug (natural [t, d] layout), per head
                    for hl in range(HPC):
                        inst = b * HPC + hl
                        for tb in range(16):
                            pt = pt_pool.tile([128, HD], F32R, tag="pt")
                            nc.tensor.transpose(
                                pt[:],
                                vT[hl * HD:(hl + 1) * HD,
                                   tb * 128:(tb + 1) * 128],
                                iden[hl * HD:(hl + 1) * HD,
                                     hl * HD:(hl + 1) * HD],
                            )
                            nc.vector.tensor_copy(
                                v_aug[inst][:, tb, 0:HD], pt[:]
                            )
                        nc.any.memset(
                            v_aug[inst][:, :, HD:HD + 1].bitcast(F32), 1.0
                        )

            # ================= Phase B: attention (pair-interleaved) ========
            with (
                tc.tile_pool(name="psc", bufs=2, space="PSUM") as psc_pool,
                tc.tile_pool(name="pmisc", bufs=2, space="PSUM") as pmisc_pool,
            ):
                for hl in range(HPC):
                    rst = {}
                    uns = {}
                    for qh in range(2):
                        q0 = qh * 1024
                        outT, exs, scps = {}, {}, {}
                        for b in range(B):
                            outT[b] = pmisc_pool.tile(
                                [HD + 1, 1024], F32, tag="pm",
                                name=f"outT{hl}{qh}{b}",
                            )
                            if qh == 0:
                                rst[b] = rstage_pool.tile(
                                    [1, T], F32, tag="rst", name=f"rst{hl}{b}"
                                )
                        for sc in range(17):
                            for b in range(B):
                                inst = b * HPC + hl
                                if sc < 16:
                                    scp = psc_pool.tile(
                                        [128, 1024], F32, tag="sc",
                                        name=f"scp{hl}{qh}{b}_{sc}",
                                    )
                                    for qg in range(2):
                                        for qpart, st in ((q_sb, True),
                                                          (q_lo, False)):
                                            nc.tensor.matmul(
                                                scp[:, qg * 512:(qg + 1) * 512],
                                                k_sb[b][hl * HD:(hl + 1) * HD,
                                                        sc * 128:(sc + 1) * 128],
                                                qpart[b][hl * HD:(hl + 1) * HD,
                                                         q0 + qg * 512:
                                                         q0 + (qg + 1) * 512],
                                                start=st, stop=not st,
                                            )
                                    scps[b] = scp
                                if sc >= 1:
                                    ex_prev = exs[(b, sc - 1)]
                                    for qg in range(2):
                                        nc.tensor.matmul(
                                            outT[b][:, qg * 512:(qg + 1) * 512],
                                            v_aug[inst][:, sc - 1, :],
                                            ex_prev[:, qg * 512:(qg + 1) * 512],
                                            start=(sc == 1), stop=(sc == 16),
                                        )
                                if sc < 16:
                                    ex = exp_pool.tile(
                                        [128, 1024], F32R, tag="ex",
                                        name=f"ex{hl}{qh}{b}_{sc}",
                                    )
                                    nc.scalar.activation(
                                        ex[:], scps[b][:], AFT.Exp
                                    )
                                    exs[(b, sc)] = ex
                        # row-sums + unnormalized out to SBUF; free psum
                        for b in range(B):
                            nc.vector.tensor_copy(
                                rst[b][:, q0:q0 + 1024], outT[b][HD:HD + 1, :]
                            )
                            un = unnorm_pool.tile(
                                [HD, 1024], F32, tag="un",
                                name=f"un{hl}{qh}{b}",
                            )
                            nc.vector.tensor_copy(un[:], outT[b][0:HD, :])
                            uns[(b, qh)] = un
                    # reciprocal + divide + stage into a2a_in[hl]
                    for b in range(B):
                        lnr = rinv_pool.tile([1, T], F32, tag="lnr",
                                             name=f"lnr{hl}{b}")
                        nc.scalar.activation(lnr[:], rst[b][:], AFT.Ln)
                        rinv = rinv_pool.tile([1, T], F32R, tag="rinv",
                                              name=f"rinv{hl}{b}")
                        nc.scalar.activation(rinv[:], lnr[:], AFT.Exp,
                                             scale=-1.0)
                        for qh in range(2):
                            q0 = qh * 1024
                            bc = psc_pool.tile([128, 1024], F32, tag="sc",
                                               name=f"bc{hl}{b}{qh}")
                            for qg in range(2):
                                nc.tensor.matmul(
                                    bc[0:HD, qg * 512:(qg + 1) * 512],
                                    ones[0:1, 0:HD],
                                    rinv[:, q0 + qg * 512:q0 + (qg + 1) * 512],
                                    start=True, stop=True,
                                )
                            sc_t = scaled_pool.tile([HD, 1024], F32R,
                                                    tag="sca",
                                                    name=f"sca{hl}{b}{qh}")
                            nc.vector.tensor_mul(
                                sc_t[:], uns[(b, qh)][:], bc[0:HD, :]
                            )
                            for half in range(2):
                                j = 4 * b + 2 * qh + half
                                nc.sync.dma_start(
                                    a2a_in[hl][j, :, :],
                                    sc_t[:, half * 512:(half + 1) * 512],
                                )
                    # launch this head's AllToAll (overlaps next head's work)
                    nc.gpsimd.collective_compute(
                        "AllToAll",
                        mybir.AluOpType.bypass,
                        replica_groups=[list(range(NC))],
                        ins=[a2a_in[hl].ap().opt()],
                        outs=[a2a_out[hl].ap().opt()],
                    )

            # ================= Phase C: out_proj =================
            a2a_flat = [
                a2a_out[h].ap().rearrange("s p r -> (s p) r")
                for h in range(HPC)
            ]
            with tc.tile_pool(name="pop", bufs=4, space="PSUM") as pop_pool:
                ops = [pop_pool.tile([128, D], F32, tag="op", name=f"op{qb}")
                       for qb in range(4)]
                for kc in range(8):
                    wo_t = woc_pool.tile([128, D], F32R, tag="woc")
                    nc.sync.dma_start(
                        wo_t[:], wo_in[kc * 128:(kc + 1) * 128, :]
                    )
                    src = a2a_flat[kc // 4]
                    r0 = (kc % 4) * 128
                    for qb in range(4):
                        lh = oplhs_pool.tile([128, 128], F32R, tag="lh")
                        nc.sync.dma_start(
                            lh[:],
                            src[r0:r0 + 128, qb * 128:(qb + 1) * 128],
                        )
                        for ng in range(2):
                            nc.tensor.matmul(
                                ops[qb][:, ng * 512:(ng + 1) * 512],
                                lh[:],
                                wo_t[:, ng * 512:(ng + 1) * 512],
                                start=(kc == 0), stop=False,
                            )
                for qb in range(4):
                    for ng in range(2):  # bias row (K=1 ones)
                        nc.tensor.matmul(
                            ops[qb][:, ng * 512:(ng + 1) * 512],
                            ones[0:1, :],
                            wob_sb[0:1, ng * 512:(ng + 1) * 512],
                            start=False, stop=True,
                        )
                    fin = fin_pool.tile([128, D], F32, tag="fin")
                    nc.vector.tensor_copy(fin[:], ops[qb][:])
                    nc.sync.dma_start(
                        out[qb * 128:(qb + 1) * 128, :], fin[:]
                    )
    nc.compile()
    return nc


def _get_nc():
    if "nc" not in _CACHE:
        _CACHE["nc"] = _build()
    return _CACHE["nc"]


def kernel(query, w_q, w_k, w_v, w_o, b_q, b_k, b_v, b_o):
    global LAST_EXEC_TIME_NS
    from concourse.bass_utils import run_bass_kernel_spmd

    query = np.asarray(query, dtype=np.float32)
    w_q = np.asarray(w_q, dtype=np.float32)
    w_k = np.asarray(w_k, dtype=np.float32)
    w_v = np.asarray(w_v, dtype=np.float32)
    w_o = np.asarray(w_o, dtype=np.float32)
    b_q = np.asarray(b_q, dtype=np.float32)
    b_k = np.asarray(b_k, dtype=np.float32)
    b_v = np.asarray(b_v, dtype=np.float32)
    b_o = np.asarray(b_o, dtype=np.float32)

    # host-side prep
    qT = np.ascontiguousarray(query.transpose(0, 2, 1))          # [B, D, T]
    # permute w_o rows (contraction dim) to the A2A output order:
    # [even-head dims of core 0..7, odd-head dims of core 0..7]
    perm = np.concatenate([
        np.concatenate([np.arange(128 * c + 64 * h, 128 * c + 64 * h + 64)
                        for c in range(NC)])
        for h in range(HPC)
    ])
    wo_t = np.ascontiguousarray(w_o.T[perm])                     # [D, D]
    wob = np.ascontiguousarray(b_o[None, :])                     # [1, D]
    iden = np.eye(128, dtype=np.float32)

    in_maps = []
    for c in range(NC):
        sl = slice(128 * c, 128 * (c + 1))
        wqkv = np.ascontiguousarray(
            np.concatenate(
                [w_q[sl] * SCALE, w_k[sl], w_v[sl]], axis=0
            ).T
        )                                                        # [D, 384]
        bqkv = np.stack(
            [b_q[sl] * SCALE, b_k[sl], b_v[sl]], axis=1
        ).astype(np.float32)                                     # [128, 3]
        in_maps.append({
            "qT": qT, "wqkv": wqkv, "bqkv": bqkv,
            "wo": wo_t, "wob": wob, "iden": iden,
        })

    nc = _get_nc()
    trace = os.environ.get("KERNEL_TRACE") == "1"
    if trace:
        _install_ntff_hook()
    if trace:
        _install_ntff_hook()
    res = run_bass_kernel_spmd(
        nc, in_maps, core_ids=list(range(NC)), trace=trace
    )
    LAST_EXEC_TIME_NS = res.exec_time_ns
    rows = np.concatenate([res.results[c]["out"] for c in range(NC)], axis=0)
    return rows.reshape(B, T, D)


# revision 5
# speedup vs baseline: 1.3688x; 1.3688x over previous
"""Multi-head self-attention (B=2, T=2048, D=1024, H=16) on 8 TRN2 NeuronCores.

Sharding: heads {2c, 2c+1} (both batches) on core c -> attention head-parallel;
two 8-core AllToAlls (one per local head) redistribute attention output from
head-slices to row-blocks; out_proj row-sharded (core c computes flattened rows
[512c, 512c+512) of the [4096, 1024] output). w_o rows are host-permuted to
match the AllToAll output dim order.

All matmuls run in float32r (TF32-like: ~1.3e-4 rel err, 4x faster than fp32).
Softmax uses the transposed-scores layout [s, q]: exp on ACT (no max
subtraction needed in fp32 range), row-sums via a ones-column appended to V in
the PV matmul, reciprocal via exp(-ln r) on ACT, normalizer applied through a
PE outer-product broadcast + DVE multiply.

Attention interleaves the two batches' instances of the same local head
(independent chains) to keep TensorE dense (avoids HAM re-throttle), with PV
lagged one s-chunk behind exp. The first AllToAll overlaps the second head's
attention.
"""
import os
import numpy as np

B, T, D, H = 2, 2048, 1024, 16
HD = D // H
SCALE = HD ** -0.5
NC = 8
HPC = H // NC          # heads per core = 2
ROWS = B * T // NC     # output rows per core = 512

LAST_EXEC_TIME_NS = None
_CACHE = {}


def _install_ntff_hook():
    """Register the axon NTFF profile hook (missing antenv.axon_hooks shim)
    so run_bass_kernel_spmd(trace=True) can return exec_time_ns."""
    import sys
    import types
    try:
        import antenv
        if "antenv.axon_hooks" in sys.modules:
            return
        mod = types.ModuleType("antenv.axon_hooks")
        state = {"hook": None}
        mod.set_axon_ntff_profile_hook = lambda h: state.__setitem__("hook", h)
        mod.get_axon_ntff_profile_hook = lambda: state["hook"]
        sys.modules["antenv.axon_hooks"] = mod
        antenv.axon_hooks = mod
        from trn_agent_boot.trn_boot import _ntff_profile_via_ctypes
        mod.set_axon_ntff_profile_hook(
            _ntff_profile_via_ctypes("/opt/axon/libaxon_pjrt.so")
        )
    except Exception:
        pass


def _install_ntff_hook():
    """Register the axon NTFF profile hook (missing antenv.axon_hooks shim)
    so run_bass_kernel_spmd(trace=True) can return exec_time_ns."""
    import sys
    import types
    try:
        import antenv
        if "antenv.axon_hooks" in sys.modules:
            return
        mod = types.ModuleType("antenv.axon_hooks")
        state = {"hook": None}
        mod.set_axon_ntff_profile_hook = lambda h: state.__setitem__("hook", h)
        mod.get_axon_ntff_profile_hook = lambda: state["hook"]
        sys.modules["antenv.axon_hooks"] = mod
        antenv.axon_hooks = mod
        from trn_agent_boot.trn_boot import _ntff_profile_via_ctypes
        mod.set_axon_ntff_profile_hook(
            _ntff_profile_via_ctypes("/opt/axon/libaxon_pjrt.so")
        )
    except Exception:
        pass


def _enable_ldw_opt():
    """Let walrus dedupe back-to-back LDWEIGHTS of the same stationary
    operand (concourse hardcodes --enable-ldw-opt=false)."""
    from concourse import bass_utils as _bu
    if getattr(_bu, "_ldw_opt_patched", False):
        return
    _orig = _bu.run_command

    def _patched(cmd, *a, **kw):
        cmd = [c.replace("--enable-ldw-opt=false", "--enable-ldw-opt=true")
               if isinstance(c, str) else c for c in cmd]
        return _orig(cmd, *a, **kw)

    _bu.run_command = _patched
    _bu._ldw_opt_patched = True


def _build():
    import concourse.bass as bass
    import concourse.tile as tile
    from concourse import bacc, mybir
    _enable_ldw_opt()

    F32 = mybir.dt.float32
    F32R = mybir.dt.float32r
    AFT = mybir.ActivationFunctionType

    nc = bacc.Bacc(
        "TRN2", target_bir_lowering=False, debug=False,
        enable_asserts=True, num_devices=NC,
    )

    # ---- I/O ----
    qT_in = nc.dram_tensor("qT", [B, D, T], F32R, kind="ExternalInput")
    wqkv_in = nc.dram_tensor("wqkv", [D, 3 * 128], F32R, kind="ExternalInput")
    bqkv_in = nc.dram_tensor("bqkv", [128, 3], F32, kind="ExternalInput")
    # wo rows permuted: [even-head dims (512), odd-head dims (512)]
    wo_in = nc.dram_tensor("wo", [D, D], F32R, kind="ExternalInput")
    wob_in = nc.dram_tensor("wob", [1, D], F32R, kind="ExternalInput")
    iden_in = nc.dram_tensor("iden", [128, 128], F32R, kind="ExternalInput")
    out = nc.dram_tensor("out", [ROWS, D], F32, kind="ExternalOutput")

    # A2A bounce buffers, one pair per local head
    a2a_in = [nc.dram_tensor(f"a2a_in{h}", [NC, HD, ROWS], F32R)
              for h in range(HPC)]
    a2a_out = [nc.dram_tensor(f"a2a_out{h}", [NC, HD, ROWS], F32R)
               for h in range(HPC)]

    with tile.TileContext(nc) as tc:
        with (
            tc.tile_pool(name="persist", bufs=1) as persist,
            tc.tile_pool(name="qrhs", bufs=3) as qrhs_pool,
            tc.tile_pool(name="vt", bufs=1) as vt_pool,
            tc.tile_pool(name="exp", bufs=5) as exp_pool,
            tc.tile_pool(name="unnorm", bufs=5) as unnorm_pool,
            tc.tile_pool(name="scaled", bufs=2) as scaled_pool,
            tc.tile_pool(name="rstage", bufs=2) as rstage_pool,
            tc.tile_pool(name="rinv", bufs=2) as rinv_pool,
            tc.tile_pool(name="oplhs", bufs=4) as oplhs_pool,
            tc.tile_pool(name="woc", bufs=2) as woc_pool,
            tc.tile_pool(name="fin", bufs=2) as fin_pool,
        ):
            # ---- persistent tiles ----
            iden = persist.tile([128, 128], F32R, tag="iden")
            nc.sync.dma_start(iden[:], iden_in[:, :])
            ones = persist.tile([1, 128], F32R, tag="ones")
            nc.any.memset(ones[:].bitcast(F32), 1.0)
            bias_sb = persist.tile([128, 3], F32, tag="bias")
            nc.sync.dma_start(bias_sb[:], bqkv_in[:, :])
            wqkv_sb = persist.tile([128, 8, 3 * 128], F32R, tag="wqkv")
            nc.sync.dma_start(
                wqkv_sb[:], wqkv_in.ap().rearrange("(kc p) m -> p kc m", p=128)
            )
            wob_sb = persist.tile([1, D], F32R, tag="wob")
            nc.sync.dma_start(wob_sb[:], wob_in[:, :])

            q_sb = [persist.tile([128, T], F32R, tag=f"q{b}", name=f"q_sb{b}")
                    for b in range(B)]
            k_sb = [persist.tile([128, T], F32R, tag=f"k{b}", name=f"k_sb{b}")
                    for b in range(B)]
            v_aug = [
                persist.tile([128, 16, HD + 1], F32R, tag=f"va{i}",
                             name=f"v_aug{i}")
                for i in range(B * HPC)
            ]

            # ================= Phase A: projections =================
            with (
                tc.tile_pool(name="px", bufs=4, space="PSUM") as px_pool,
                tc.tile_pool(name="pt", bufs=2, space="PSUM") as pt_pool,
            ):
                for b in range(B):
                    vT = vt_pool.tile([128, T], F32R, tag="vt")
                    for nb in range(4):
                        ps = [
                            px_pool.tile([128, 512], F32, tag="px",
                                         name=f"ps{b}_{nb}_{mb}")
                            for mb in range(3)
                        ]
                        for kc in range(8):
                            qr = qrhs_pool.tile([128, 512], F32R, tag="qr")
                            nc.sync.dma_start(
                                qr[:],
                                qT_in[b, kc * 128:(kc + 1) * 128,
                                      nb * 512:(nb + 1) * 512],
                            )
                            for mb in range(3):
                                nc.tensor.matmul(
                                    ps[mb][:],
                                    wqkv_sb[:, kc, mb * 128:(mb + 1) * 128],
                                    qr[:],
                                    start=(kc == 0), stop=(kc == 7),
                                )
                        dests = [q_sb[b], k_sb[b], vT]
                        for mb in range(3):
                            nc.vector.tensor_scalar_add(
                                dests[mb][:, nb * 512:(nb + 1) * 512],
                                ps[mb][:],
                                bias_sb[:, mb:mb + 1],
                            )
                    # transpose vT -> v_aug (natural [t, d] layout), per head
                    for hl in range(HPC):
                        inst = b * HPC + hl
                        for tb in range(16):
                            pt = pt_pool.tile([128, HD], F32R, tag="pt")
                            nc.tensor.transpose(
                                pt[:],
                                vT[hl * HD:(hl + 1) * HD,
                                   tb * 128:(tb + 1) * 128],
                                iden[hl * HD:(hl + 1) * HD,
                                     hl * HD:(hl + 1) * HD],
                            )
                            nc.vector.tensor_copy(
                                v_aug[inst][:, tb, 0:HD], pt[:]
                            )
                        nc.any.memset(
                            v_aug[inst][:, :, HD:HD + 1].bitcast(F32), 1.0
                        )

            # ================= Phase B: attention (pair-interleaved) ========
            with (
                tc.tile_pool(name="psc", bufs=2, space="PSUM") as psc_pool,
                tc.tile_pool(name="pmisc", bufs=2, space="PSUM") as pmisc_pool,
            ):
                for hl in range(HPC):
                    rst = {}
                    uns = {}
                    for qh in range(2):
                        q0 = qh * 1024
                        outT, exs, scps = {}, {}, {}
                        for b in range(B):
                            outT[b] = pmisc_pool.tile(
                                [HD + 1, 1024], F32, tag="pm",
                                name=f"outT{hl}{qh}{b}",
                            )
                            if qh == 0:
                                rst[b] = rstage_pool.tile(
                                    [1, T], F32, tag="rst", name=f"rst{hl}{b}"
                                )
                        for sc in range(17):
                            for b in range(B):
                                inst = b * HPC + hl
                                if sc < 16:
                                    scp = psc_pool.tile(
                                        [128, 1024], F32, tag="sc",
                                        name=f"scp{hl}{qh}{b}_{sc}",
                                    )
                                    for qg in range(2):
                                        nc.tensor.matmul(
                                            scp[:, qg * 512:(qg + 1) * 512],
                                            k_sb[b][hl * HD:(hl + 1) * HD,
                                                    sc * 128:(sc + 1) * 128],
                                            q_sb[b][hl * HD:(hl + 1) * HD,
                                                    q0 + qg * 512:
                                                    q0 + (qg + 1) * 512],
                                            start=True, stop=True,
                                        )
                                    scps[b] = scp
                                if sc >= 1:
                                    ex_prev = exs[(b, sc - 1)]
                                    for qg in range(2):
                                        nc.tensor.matmul(
                                            outT[b][:, qg * 512:(qg + 1) * 512],
                                            v_aug[inst][:, sc - 1, :],
                                            ex_prev[:, qg * 512:(qg + 1) * 512],
                                            start=(sc == 1), stop=(sc == 16),
                                        )
                                if sc < 16:
                                    ex = exp_pool.tile(
                                        [128, 1024], F32R, tag="ex",
                                        name=f"ex{hl}{qh}{b}_{sc}",
                                    )
                                    nc.scalar.activation(
                                        ex[:], scps[b][:], AFT.Exp
                                    )
                                    exs[(b, sc)] = ex
                        # row-sums + unnormalized out to SBUF; free psum
                        for b in range(B):
                            nc.vector.tensor_copy(
                                rst[b][:, q0:q0 + 1024], outT[b][HD:HD + 1, :]
                            )
                            un = unnorm_pool.tile(
                                [HD, 1024], F32, tag="un",
                                name=f"un{hl}{qh}{b}",
                            )
                            nc.vector.tensor_copy(un[:], outT[b][0:HD, :])
                            uns[(b, qh)] = un
                    # reciprocal + divide + stage into a2a_in[hl]
                    for b in range(B):
                        lnr = rinv_pool.tile([1, T], F32, tag="lnr",
                                             name=f"lnr{hl}{b}")
                        nc.scalar.activation(lnr[:], rst[b][:], AFT.Ln)
                        rinv = rinv_pool.tile([1, T], F32R, tag="rinv",
                                              name=f"rinv{hl}{b}")
                        nc.scalar.activation(rinv[:], lnr[:], AFT.Exp,
                                             scale=-1.0)
                        for qh in range(2):
                            q0 = qh * 1024
                            bc = psc_pool.tile([128, 1024], F32, tag="sc",
                                               name=f"bc{hl}{b}{qh}")
                            for qg in range(2):
                                nc.tensor.matmul(
                                    bc[0:HD, qg * 512:(qg + 1) * 512],
                                    ones[0:1, 0:HD],
                                    rinv[:, q0 + qg * 512:q0 + (qg + 1) * 512],
                                    start=True, stop=True,
                                )
                            sc_t = scaled_pool.tile([HD, 1024], F32R,
                                                    tag="sca",
                                                    name=f"sca{hl}{b}{qh}")
                            nc.vector.tensor_mul(
                                sc_t[:], uns[(b, qh)][:], bc[0:HD, :]
                            )
                            for half in range(2):
                                j = 4 * b + 2 * qh + half
                                nc.sync.dma_start(
                                    a2a_in[hl][j, :, :],
                                    sc_t[:, half * 512:(half + 1) * 512],
                                )
                    # launch this head's AllToAll (overlaps next head's work)
                    nc.gpsimd.collective_compute(
                        "AllToAll",
                        mybir.AluOpType.bypass,
                        replica_groups=[list(range(NC))],
                        ins=[a2a_in[hl].ap().opt()],
                        outs=[a2a_out[hl].ap().opt()],
                    )

            # ================= Phase C: out_proj =================
            a2a_flat = [
                a2a_out[h].ap().rearrange("s p r -> (s p) r")
                for h in range(HPC)
            ]
            with tc.tile_pool(name="pop", bufs=4, space="PSUM") as pop_pool:
                ops = [pop_pool.tile([128, D], F32, tag="op", name=f"op{qb}")
                       for qb in range(4)]
                for kc in range(8):
                    wo_t = woc_pool.tile([128, D], F32R, tag="woc")
                    nc.sync.dma_start(
                        wo_t[:], wo_in[kc * 128:(kc + 1) * 128, :]
                    )
                    src = a2a_flat[kc // 4]
                    r0 = (kc % 4) * 128
                    for qb in range(4):
                        lh = oplhs_pool.tile([128, 128], F32R, tag="lh")
                        nc.sync.dma_start(
                            lh[:],
                            src[r0:r0 + 128, qb * 128:(qb + 1) * 128],
                        )
                        for ng in range(2):
                            nc.tensor.matmul(
                                ops[qb][:, ng * 512:(ng + 1) * 512],
                                lh[:],
                                wo_t[:, ng * 512:(ng + 1) * 512],
                                start=(kc == 0), stop=False,
                            )
                for qb in range(4):
                    for ng in range(2):  # bias row (K=1 ones)
                        nc.tensor.matmul(
                            ops[qb][:, ng * 512:(ng + 1) * 512],
                            ones[0:1, :],
                            wob_sb[0:1, ng * 512:(ng + 1) * 512],
                            start=False, stop=True,
                        )
                    fin = fin_pool.tile([128, D], F32, tag="fin")
                    nc.vector.tensor_copy(fin[:], ops[qb][:])
                    nc.sync.dma_start(
                        out[qb * 128:(qb + 1) * 128, :], fin[:]
                    )
    nc.compile()
    return nc


def _get_nc():
    if "nc" not in _CACHE:
        _CACHE["nc"] = _build()
    return _CACHE["nc"]


def kernel(query, w_q, w_k, w_v, w_o, b_q, b_k, b_v, b_o):
    global LAST_EXEC_TIME_NS
    from concourse.bass_utils import run_bass_kernel_spmd

    query = np.asarray(query, dtype=np.float32)
    w_q = np.asarray(w_q, dtype=np.float32)
    w_k = np.asarray(w_k, dtype=np.float32)
    w_v = np.asarray(w_v, dtype=np.float32)
    w_o = np.asarray(w_o, dtype=np.float32)
    b_q = np.asarray(b_q, dtype=np.float32)
    b_k = np.asarray(b_k, dtype=np.float32)
    b_v = np.asarray(b_v, dtype=np.float32)
    b_o = np.asarray(b_o, dtype=np.float32)

    # host-side prep
    qT = np.ascontiguousarray(query.transpose(0, 2, 1))          # [B, D, T]
    # permute w_o rows (contraction dim) to the A2A output order:
    # [even-head dims of core 0..7, odd-head dims of core 0..7]
    perm = np.concatenate([
        np.concatenate([np.arange(128 * c + 64 * h, 128 * c + 64 * h + 64)
                        for c in range(NC)])
        for h in range(HPC)
    ])
    wo_t = np.ascontiguousarray(w_o.T[perm])                     # [D, D]
    wob = np.ascontiguousarray(b_o[None, :])                     # [1, D]
    iden = np.eye(128, dtype=np.float32)

    in_maps = []
    for c in range(NC):
        sl = slice(128 * c, 128 * (c + 1))
        wqkv = np.ascontiguousarray(
            np.concatenate(
                [w_q[sl] * SCALE, w_k[sl], w_v[sl]], axis=0
            ).T
        )                                                        # [D, 384]
        bqkv = np.stack(
            [b_q[sl] * SCALE, b_k[sl], b_v[sl]], axis=1
        ).astype(np.float32)                                     # [128, 3]
        in_maps.append({
            "qT": qT, "wqkv": wqkv, "bqkv": bqkv,
            "wo": wo_t, "wob": wob, "iden": iden,
        })

    nc = _get_nc()
    trace = os.environ.get("KERNEL_TRACE") == "1"
    if trace:
        _install_ntff_hook()
    if trace:
        _install_ntff_hook()
    res = run_bass_kernel_spmd(
        nc, in_maps, core_ids=list(range(NC)), trace=trace
    )
    LAST_EXEC_TIME_NS = res.exec_time_ns
    rows = np.concatenate([res.results[c]["out"] for c in range(NC)], axis=0)
    return rows.reshape(B, T, D)
